# revision 1
# baseline (speedup 1.0000x reference)
"""Trainium2 Bass kernel for masked-dropout attention-score matmul.

Computes, for q/k/v [B,H,S,D] and an int32 0/1 keep-mask [B,H,S,S]:

    out = ((q @ k^T) * sqrt(D) * 2 * mask) @ v        (2 = 1/(1-p_drop))

Strategy (8 NeuronCores, SPMD, no collectives):
  - Shard the 32 (b,h) pairs 4-per-core.
  - Per pair, compute S^T = K @ Q^T on the PE (so the second matmul can
    consume it as its moving operand without any on-chip transpose),
    apply the mask fused into the PSUM->SBUF eviction on the DVE, and
    accumulate O^T = V^T @ S'^T on the PE.
  - The scale (2*sqrt(D)) is folded into V on the host; mask values are
    shipped as fp8(0/1) bytes; Q^T/K^T/V are host-rearranged so all
    device DMAs are fully contiguous.
"""

import os
import sys

sys.path.insert(0, "/opt/trn_rl_repo")

import numpy as np

import concourse.bacc as bacc
import concourse.bass as bass
import concourse.mybir as mybir
import concourse.tile as tile
from concourse.bass_utils import run_bass_kernel_spmd

B, H, SQ, SK, D = 2, 16, 2048, 2048, 128
P_DROP = 0.5
SCALE = float(D) ** 0.5 / (1.0 - P_DROP)  # folded into V on the host
N_CORES = 8
PAIRS = B * H
PAIRS_PER_CORE = PAIRS // N_CORES

F32 = mybir.dt.float32
F32R = mybir.dt.float32r
FP8 = mybir.dt.float8e4
U8 = mybir.dt.uint8
BF16 = mybir.dt.bfloat16

FP8_ONE = 0x38  # float8_e4m3 encoding of 1.0

# module-level handle for test.py to inspect timing after a traced run
LAST_RESULTS = None


def emit_body(nc, tc, ot, qt, kt, v, mt, n_pairs, sq, sk, d=D, qn=512, repeat=1,
              loop_n=1, mmdt=F32R):
    """Emit the per-core program.

    APs (all on this core's DRAM):
      qt [n_pairs, d,  sq]  f32  : Q^T per pair
      kt [n_pairs, d,  sk]  f32  : K^T per pair
      v  [n_pairs, d?, ...]      : V rearranged to [128, (sk//128)*d], f32,
                                   v[p][r][c*d+j] = V[c*128+r, j] * SCALE
      mt [n_pairs, sk, sq]  u8   : mask^T as fp8 bytes (0x00 / 0x38)
      ot [n_pairs, d,  sq]  f32  : O^T output
    """
    nkc = sk // 128
    nqc = sq // qn

    import contextlib

    with contextlib.ExitStack() as ctx:
        qt_pool = ctx.enter_context(tc.tile_pool(name="qt", bufs=2))
        kt_pool = ctx.enter_context(tc.tile_pool(name="kt", bufs=2))
        v_pool = ctx.enter_context(tc.tile_pool(name="v", bufs=2))
        m_pool = ctx.enter_context(tc.tile_pool(name="m", bufs=4))
        sp_pool = ctx.enter_context(tc.tile_pool(name="sp", bufs=6))
        o_pool = ctx.enter_context(tc.tile_pool(name="o", bufs=2))
        st_pool = ctx.enter_context(tc.tile_pool(name="st", bufs=4, space="PSUM"))
        ot_pool = ctx.enter_context(tc.tile_pool(name="otp", bufs=1, space="PSUM"))

        loop_cm = tc.For_i(0, loop_n, 1) if loop_n > 1 else contextlib.nullcontext()
        with loop_cm:
          for p in [pp for _ in range(repeat) for pp in range(n_pairs)]:
            qt_t = qt_pool.tile([128, sq], mmdt)
            nc.sync.dma_start(out=qt_t[:d], in_=qt[p])
            kt_t = kt_pool.tile([128, sk], mmdt)
            nc.sync.dma_start(out=kt_t[:d], in_=kt[p])
            v_t = v_pool.tile([128, nkc * d], mmdt)
            nc.sync.dma_start(out=v_t[:], in_=v[p])

            ot_ps = ot_pool.tile([128, sq], F32)

            for kc in range(nkc):
                m_t = m_pool.tile([128, sq], U8)
                nc.sync.dma_start(out=m_t[:], in_=mt[p, kc * 128 : (kc + 1) * 128, :])

                for qc in range(nqc):
                    st = st_pool.tile([128, qn], F32)
                    nc.tensor.matmul(
                        st[:],
                        kt_t[:d, kc * 128 : (kc + 1) * 128],
                        qt_t[:d, qc * qn : (qc + 1) * qn],
                        start=True,
                        stop=True,
                    )
                    sp = sp_pool.tile([128, qn], mmdt)
                    nc.vector.tensor_mul(
                        sp[:],
                        st[:],
                        m_t[:, qc * qn : (qc + 1) * qn].bitcast(FP8),
                    )
                    nc.tensor.matmul(
                        ot_ps[:d, qc * qn : (qc + 1) * qn],
                        v_t[:, kc * d : (kc + 1) * d],
                        sp[:],
                        start=(kc == 0),
                        stop=(kc == nkc - 1),
                    )

            o_t = o_pool.tile([128, sq], F32)
            nc.scalar.copy(o_t[:d], ot_ps[:d])
            nc.sync.dma_start(out=ot[p], in_=o_t[:d])


def emit_body_v2(
    nc, tc, ot, qt, kt, v, mt, n_pairs, sq, sk, d=D, qn=512, gn=1024, fused_mod=(1, 4),
    repeat=1, loop_n=1,
):
    """Balanced-engine variant.

    Masking is split across three engines per [128, gn] score group:
      - fused path (idx % fused_mod[1] < fused_mod[0]): DVE multiplies
        PSUM f32 scores by the fp8 mask directly -> bf16 SBUF.
      - split path: ACT evicts PSUM f32 -> bf16 SBUF, GpSimd converts the
        fp8 mask -> bf16, DVE multiplies bf16 x bf16 in its 2x mode.
    Second matmul runs with bf16 moving operand at N=gn; V ships as bf16.
    """
    nkc = sk // 128
    ngc = sq // gn

    import contextlib

    with contextlib.ExitStack() as ctx:
        qt_pool = ctx.enter_context(tc.tile_pool(name="qt", bufs=2))
        kt_pool = ctx.enter_context(tc.tile_pool(name="kt", bufs=2))
        v_pool = ctx.enter_context(tc.tile_pool(name="v", bufs=2))
        m_pool = ctx.enter_context(tc.tile_pool(name="m", bufs=4))
        sp_pool = ctx.enter_context(tc.tile_pool(name="sp", bufs=6))
        se_pool = ctx.enter_context(tc.tile_pool(name="se", bufs=4))
        mb_pool = ctx.enter_context(tc.tile_pool(name="mb", bufs=4))
        o_pool = ctx.enter_context(tc.tile_pool(name="o", bufs=2))
        st_pool = ctx.enter_context(tc.tile_pool(name="st", bufs=2, space="PSUM"))
        ot_pool = ctx.enter_context(tc.tile_pool(name="otp", bufs=1, space="PSUM"))

        unit = 0
        loop_cm = tc.For_i(0, loop_n, 1) if loop_n > 1 else contextlib.nullcontext()
        with loop_cm:
          for p in [pp for _ in range(repeat) for pp in range(n_pairs)]:
            qt_t = qt_pool.tile([128, sq], F32R)
            nc.sync.dma_start(out=qt_t[:d], in_=qt[p])
            kt_t = kt_pool.tile([128, sk], F32R)
            nc.sync.dma_start(out=kt_t[:d], in_=kt[p])
            v_t = v_pool.tile([128, nkc * d], BF16)
            nc.sync.dma_start(out=v_t[:], in_=v[p])

            ot_ps = ot_pool.tile([128, sq], F32)

            for kc in range(nkc):
                m_t = m_pool.tile([128, sq], U8)
                nc.sync.dma_start(out=m_t[:], in_=mt[p, kc * 128 : (kc + 1) * 128, :])

                for g in range(ngc):
                    st = st_pool.tile([128, gn], F32)
                    for j in range(gn // qn):
                        c0 = g * gn + j * qn
                        nc.tensor.matmul(
                            st[:, j * qn : (j + 1) * qn],
                            kt_t[:d, kc * 128 : (kc + 1) * 128],
                            qt_t[:d, c0 : c0 + qn],
                            start=True,
                            stop=True,
                        )
                    m_sl = m_t[:, g * gn : (g + 1) * gn].bitcast(FP8)
                    sp = sp_pool.tile([128, gn], BF16)
                    if unit % fused_mod[1] < fused_mod[0]:
                        nc.vector.tensor_mul(sp[:], st[:], m_sl)
                    else:
                        se = se_pool.tile([128, gn], BF16)
                        nc.scalar.copy(se[:], st[:])
                        mb = mb_pool.tile([128, gn], BF16)
                        nc.gpsimd.tensor_copy(mb[:], m_sl)
                        nc.vector.tensor_mul(sp[:], se[:], mb[:])
                    unit += 1
                    for j in range(gn // qn):
                        c0 = g * gn + j * qn
                        nc.tensor.matmul(
                            ot_ps[:d, c0 : c0 + qn],
                            v_t[:, kc * d : (kc + 1) * d],
                            sp[:, j * qn : (j + 1) * qn],
                            start=(kc == 0),
                            stop=(kc == nkc - 1),
                        )

            o_t = o_pool.tile([128, sq], F32)
            nc.scalar.copy(o_t[:d], ot_ps[:d])
            nc.sync.dma_start(out=ot[p], in_=o_t[:d])


def build_nc(n_pairs=PAIRS_PER_CORE, sq=SQ, sk=SK, d=D, qn=512, variant="v1", repeat=1,
             loop_n=1):
    nc = bacc.Bacc("TRN2", target_bir_lowering=False, debug=False)
    mmdt = F32R if variant == "v1" else BF16
    vdt = mmdt
    qt = nc.declare_dram_parameter("qt", [n_pairs, d, sq], mmdt, isOutput=False)
    kt = nc.declare_dram_parameter("kt", [n_pairs, d, sk], mmdt, isOutput=False)
    v = nc.declare_dram_parameter("v", [n_pairs, 128, (sk // 128) * d], vdt, isOutput=False)
    mt = nc.declare_dram_parameter("mt", [n_pairs, sk, sq], U8, isOutput=False)
    ot = nc.declare_dram_parameter("ot", [n_pairs, d, sq], F32, isOutput=True)
    with tile.TileContext(nc) as tc:
        if variant in ("v1", "v3"):
            emit_body(nc, tc, ot, qt, kt, v, mt, n_pairs, sq, sk, d, qn, repeat=repeat,
                      loop_n=loop_n, mmdt=mmdt)
        else:
            emit_body_v2(nc, tc, ot, qt, kt, v, mt, n_pairs, sq, sk, d, qn,
                         repeat=repeat, loop_n=loop_n)
    nc.compile()
    return nc


def _prep_inputs(query, key, value, dropout_mask, variant="v1"):
    """Host-side marshaling into per-core input maps."""
    import ml_dtypes

    q = np.asarray(query, dtype=np.float32).reshape(PAIRS, SQ, D)
    k = np.asarray(key, dtype=np.float32).reshape(PAIRS, SK, D)
    vv = np.asarray(value, dtype=np.float32).reshape(PAIRS, SK, D)
    m = np.asarray(dropout_mask).reshape(PAIRS, SQ, SK)

    qt = np.ascontiguousarray(q.transpose(0, 2, 1))  # [PAIRS, D, SQ]
    kt = np.ascontiguousarray(k.transpose(0, 2, 1))  # [PAIRS, D, SK]
    # V * SCALE rearranged: vr[p][r][c*D+j] = V[c*128+r, j] * SCALE
    vr = (vv * np.float32(SCALE)).reshape(PAIRS, SK // 128, 128, D)
    vr = np.ascontiguousarray(vr.transpose(0, 2, 1, 3)).reshape(PAIRS, 128, (SK // 128) * D)
    if variant != "v1":
        vr = vr.astype(ml_dtypes.bfloat16)
        qt = qt.astype(ml_dtypes.bfloat16)
        kt = kt.astype(ml_dtypes.bfloat16)
    # mask^T as fp8 bytes
    mb = (m != 0).astype(np.uint8) * np.uint8(FP8_ONE)  # [PAIRS, SQ, SK] u8
    mbt = np.ascontiguousarray(mb.transpose(0, 2, 1))  # [PAIRS, SK, SQ]

    in_maps = []
    for c in range(N_CORES):
        s = slice(c * PAIRS_PER_CORE, (c + 1) * PAIRS_PER_CORE)
        in_maps.append(
            {
                "qt": qt[s],
                "kt": kt[s],
                "v": vr[s],
                "mt": mbt[s],
            }
        )
    return in_maps


def kernel(query, key, value, dropout_mask):
    global LAST_RESULTS
    variant = os.environ.get("KERNEL_VARIANT", "v1")
    in_maps = _prep_inputs(query, key, value, dropout_mask, variant)
    nc = build_nc(variant=variant)
    res = run_bass_kernel_spmd(nc, in_maps, list(range(N_CORES)), trace=False)
    LAST_RESULTS = res
    outs = np.concatenate([r["ot"] for r in res.results], axis=0)  # [PAIRS, D, SQ]
    out = outs.transpose(0, 2, 1).reshape(B, H, SQ, D)
    return np.ascontiguousarray(out.astype(np.float32, copy=False))



# revision 16
# speedup vs baseline: 2.2210x; 2.2210x over previous
"""Trainium2 Bass kernel for masked-dropout attention-score matmul.

Computes, for q/k/v [B,H,S,D] and an int32 0/1 keep-mask [B,H,S,S]:

    out = ((q @ k^T) * sqrt(D) * 2 * mask) @ v        (2 = 1/(1-p_drop))

Strategy (8 NeuronCores, SPMD, no collectives):
  - Shard the 32 (b,h) pairs 4-per-core.
  - Per pair, compute S^T = K @ Q^T on the PE (so the second matmul can
    consume it as its moving operand without any on-chip transpose),
    apply the mask fused into the PSUM->SBUF eviction on the DVE, and
    accumulate O^T = V^T @ S'^T on the PE.
  - The scale (2*sqrt(D)) is folded into V on the host; mask values are
    shipped as fp8(0/1) bytes; Q^T/K^T/V are host-rearranged so all
    device DMAs are fully contiguous.

Measured engine rates (loop-slope microbenchmarks, sustained):
  PE matmul 512-row bf16: 139 ns same-stationary / 186 ns alternating
  DVE mul f32(PSUM) x fp8 -> bf16: 1.195 ns/elem/lane (1x, the v1 evict)
  DVE mul bf16 x bf16 -> bf16 SBUF: 0.573 (2x mode)
  ACT copy PSUM f32 -> bf16: 2.182 (2.6x worse than the cost model)
  Pool mul bf16 x fp8: 1.985 raw, but in-kernel sync overheads make any
    Pool-path variant strictly slower -- do not route eviction via gpsimd.
Sustained (257-iter slope) the kernel is eviction-bound: v1 = 181 us,
v4_n8_f1_1 (bf16 MMs, eviction split 50/50 DVE-fused vs ACT-evict +
DVE-2x with host-shipped bf16 masks) = 155 us. In short bursts (the
17-iter slope printed by test.py) engine clocks boost ~1.5x and v1
measures ~103-121 us.
"""

import os
import sys

sys.path.insert(0, "/opt/trn_rl_repo")

import numpy as np

import concourse.bacc as bacc
import concourse.bass as bass
import concourse.mybir as mybir
import concourse.tile as tile
from concourse.bass_utils import run_bass_kernel_spmd

B, H, SQ, SK, D = 2, 16, 2048, 2048, 128
P_DROP = 0.5
SCALE = float(D) ** 0.5 / (1.0 - P_DROP)  # folded into V on the host
N_CORES = 8
PAIRS = B * H
PAIRS_PER_CORE = PAIRS // N_CORES

F32 = mybir.dt.float32
F32R = mybir.dt.float32r
FP8 = mybir.dt.float8e4
U8 = mybir.dt.uint8
BF16 = mybir.dt.bfloat16

FP8_ONE = 0x38  # float8_e4m3 encoding of 1.0

# module-level handle for test.py to inspect timing after a traced run
LAST_RESULTS = None

# Fastest known-correct variant; overridable via KERNEL_VARIANT for experiments.
DEFAULT_VARIANT = "v1"


def emit_body(nc, tc, ot, qt, kt, v, mt, n_pairs, sq, sk, d=D, qn=512, repeat=1,
              loop_n=1, mmdt=F32R):
    """Emit the per-core program.

    APs (all on this core's DRAM):
      qt [n_pairs, d,  sq]  f32  : Q^T per pair
      kt [n_pairs, d,  sk]  f32  : K^T per pair
      v  [n_pairs, d?, ...]      : V rearranged to [128, (sk//128)*d], f32,
                                   v[p][r][c*d+j] = V[c*128+r, j] * SCALE
      mt [n_pairs, sk, sq]  u8   : mask^T as fp8 bytes (0x00 / 0x38)
      ot [n_pairs, d,  sq]  f32  : O^T output
    """
    nkc = sk // 128
    nqc = sq // qn

    import contextlib

    with contextlib.ExitStack() as ctx:
        qt_pool = ctx.enter_context(tc.tile_pool(name="qt", bufs=2))
        kt_pool = ctx.enter_context(tc.tile_pool(name="kt", bufs=2))
        v_pool = ctx.enter_context(tc.tile_pool(name="v", bufs=2))
        m_pool = ctx.enter_context(tc.tile_pool(name="m", bufs=4))
        sp_pool = ctx.enter_context(tc.tile_pool(name="sp", bufs=6))
        o_pool = ctx.enter_context(tc.tile_pool(name="o", bufs=2))
        st_pool = ctx.enter_context(tc.tile_pool(name="st", bufs=4, space="PSUM"))
        ot_pool = ctx.enter_context(tc.tile_pool(name="otp", bufs=1, space="PSUM"))

        loop_cm = tc.For_i(0, loop_n, 1) if loop_n > 1 else contextlib.nullcontext()
        with loop_cm:
          for p in [pp for _ in range(repeat) for pp in range(n_pairs)]:
            qt_t = qt_pool.tile([128, sq], mmdt)
            nc.sync.dma_start(out=qt_t[:d], in_=qt[p])
            kt_t = kt_pool.tile([128, sk], mmdt)
            nc.sync.dma_start(out=kt_t[:d], in_=kt[p])
            v_t = v_pool.tile([128, nkc * d], mmdt)
            nc.sync.dma_start(out=v_t[:], in_=v[p])

            ot_ps = ot_pool.tile([128, sq], F32)

            for kc in range(nkc):
                m_t = m_pool.tile([128, sq], U8)
                nc.sync.dma_start(out=m_t[:], in_=mt[p, kc * 128 : (kc + 1) * 128, :])

                for qc in range(nqc):
                    st = st_pool.tile([128, qn], F32)
                    nc.tensor.matmul(
                        st[:],
                        kt_t[:d, kc * 128 : (kc + 1) * 128],
                        qt_t[:d, qc * qn : (qc + 1) * qn],
                        start=True,
                        stop=True,
                    )
                    sp = sp_pool.tile([128, qn], mmdt)
                    nc.vector.tensor_mul(
                        sp[:],
                        st[:],
                        m_t[:, qc * qn : (qc + 1) * qn].bitcast(FP8),
                    )
                    nc.tensor.matmul(
                        ot_ps[:d, qc * qn : (qc + 1) * qn],
                        v_t[:, kc * d : (kc + 1) * d],
                        sp[:],
                        start=(kc == 0),
                        stop=(kc == nkc - 1),
                    )

            o_t = o_pool.tile([128, sq], F32)
            nc.scalar.copy(o_t[:d], ot_ps[:d])
            nc.sync.dma_start(out=ot[p], in_=o_t[:d])


def emit_body_v3(nc, tc, ot, qt, kt, v, mt, n_pairs, sq, sk, d=D, qn=1024, repeat=1,
                 loop_n=1, mmdt=F32R, spdt=None):
    """Grouped-stationary variant: per kc, emit all MM1s then all MM2s so the
    PE reloads stationary weights only twice per kc (K-block, V-block)
    instead of 2*nqc times.
    """
    nkc = sk // 128
    nqc = sq // qn
    if spdt is None:
        spdt = BF16 if mmdt == BF16 else F32R

    import contextlib

    st_banks = (qn * 4) // 2048  # PSUM banks per st tile
    st_bufs = max(1, 4 // st_banks)

    with contextlib.ExitStack() as ctx:
        qt_pool = ctx.enter_context(tc.tile_pool(name="qt", bufs=2))
        kt_pool = ctx.enter_context(tc.tile_pool(name="kt", bufs=2))
        v_pool = ctx.enter_context(tc.tile_pool(name="v", bufs=2))
        m_pool = ctx.enter_context(tc.tile_pool(name="m", bufs=4))
        sp_pool = ctx.enter_context(tc.tile_pool(name="sp", bufs=2 * nqc))
        o_pool = ctx.enter_context(tc.tile_pool(name="o", bufs=2))
        st_pool = ctx.enter_context(tc.tile_pool(name="st", bufs=st_bufs, space="PSUM"))
        ot_pool = ctx.enter_context(tc.tile_pool(name="otp", bufs=1, space="PSUM"))

        loop_cm = tc.For_i(0, loop_n, 1) if loop_n > 1 else contextlib.nullcontext()
        with loop_cm:
          for p in [pp for _ in range(repeat) for pp in range(n_pairs)]:
            qt_t = qt_pool.tile([128, sq], mmdt)
            nc.sync.dma_start(out=qt_t[:d], in_=qt[p])
            kt_t = kt_pool.tile([128, sk], mmdt)
            nc.sync.dma_start(out=kt_t[:d], in_=kt[p])
            v_t = v_pool.tile([128, nkc * d], mmdt)
            nc.sync.dma_start(out=v_t[:], in_=v[p])

            ot_ps = ot_pool.tile([128, sq], F32)

            for kc in range(nkc):
                m_t = m_pool.tile([128, sq], U8)
                nc.sync.dma_start(out=m_t[:], in_=mt[p, kc * 128 : (kc + 1) * 128, :])

                sts = []
                for qc in range(nqc):
                    st = st_pool.tile([128, qn], F32)
                    nc.tensor.matmul(
                        st[:],
                        kt_t[:d, kc * 128 : (kc + 1) * 128],
                        qt_t[:d, qc * qn : (qc + 1) * qn],
                        start=True,
                        stop=True,
                    )
                    sts.append(st)
                sps = []
                for qc in range(nqc):
                    sp = sp_pool.tile([128, qn], spdt)
                    nc.vector.tensor_mul(
                        sp[:],
                        sts[qc][:],
                        m_t[:, qc * qn : (qc + 1) * qn].bitcast(FP8),
                    )
                    sps.append(sp)
                for qc in range(nqc):
                    nc.tensor.matmul(
                        ot_ps[:d, qc * qn : (qc + 1) * qn],
                        v_t[:, kc * d : (kc + 1) * d],
                        sps[qc][:],
                        start=(kc == 0),
                        stop=(kc == nkc - 1),
                    )

            o_t = o_pool.tile([128, sq], F32)
            nc.scalar.copy(o_t[:d], ot_ps[:d])
            nc.sync.dma_start(out=ot[p], in_=o_t[:d])


def emit_body_v4(nc, tc, ot, qt, kt, v, mt8, mt16, n_pairs, sq, sk, d=D,
                 n16=4, fa=4, fb=7, repeat=1, loop_n=1, depth=1):
    """Balanced-eviction variant (bf16 matmuls, bf16 output).

    The masked PSUM->SBUF eviction (the sustained bottleneck in v1/v3) is
    split three ways per [128, 1024] half-kc unit:
      - kc < n16: ACT evicts f32->bf16, DVE multiplies by a host-supplied
        bf16 mask in its 2x mode.
      - else unit%fb < fa: DVE fused multiply straight from PSUM (1x) with
        the fp8 mask.
      - else: ACT evicts, Pool (gpsimd) multiplies by the fp8 mask.
    MM2 groups are software-pipelined one kc behind MM1 groups so eviction
    latency never stalls the PE; stationary weights reload only twice per kc.
    """
    nkc = sk // 128
    un = 1024  # eviction unit width
    nu = sq // un  # units per kc (2)
    nj = un // 512  # matmuls per unit

    import contextlib

    with contextlib.ExitStack() as ctx:
        qt_pool = ctx.enter_context(tc.tile_pool(name="qt", bufs=2))
        kt_pool = ctx.enter_context(tc.tile_pool(name="kt", bufs=2))
        v_pool = ctx.enter_context(tc.tile_pool(name="v", bufs=2))
        m_pool = ctx.enter_context(tc.tile_pool(name="m", bufs=4))
        se_pool = ctx.enter_context(tc.tile_pool(name="se", bufs=4))
        sp_pool = ctx.enter_context(tc.tile_pool(name="sp", bufs=(depth + 2) * nu))
        o_pool = ctx.enter_context(tc.tile_pool(name="o", bufs=2))
        st_pool = ctx.enter_context(tc.tile_pool(name="st", bufs=2, space="PSUM"))
        ot_pool = ctx.enter_context(tc.tile_pool(name="otp", bufs=1, space="PSUM"))

        unit = 0
        loop_cm = tc.For_i(0, loop_n, 1) if loop_n > 1 else contextlib.nullcontext()
        with loop_cm:
          for p in [pp for _ in range(repeat) for pp in range(n_pairs)]:
            qt_t = qt_pool.tile([128, sq], BF16)
            nc.sync.dma_start(out=qt_t[:d], in_=qt[p])
            kt_t = kt_pool.tile([128, sk], BF16)
            nc.sync.dma_start(out=kt_t[:d], in_=kt[p])
            v_t = v_pool.tile([128, nkc * d], BF16)
            nc.sync.dma_start(out=v_t[:], in_=v[p])

            ot_ps = ot_pool.tile([128, sq], F32)

            hist = []  # pending (kc, sps) awaiting their MM2 group
            for kc in range(nkc):
                if kc < n16:
                    m_t = m_pool.tile([128, sq], BF16)
                    nc.sync.dma_start(
                        out=m_t[:], in_=mt16[p, kc * 128 : (kc + 1) * 128, :]
                    )
                else:
                    m_t = m_pool.tile([128, sq], U8)
                    nc.sync.dma_start(
                        out=m_t[:], in_=mt8[p, kc * 128 : (kc + 1) * 128, :]
                    )

                # MM1 group for kc
                sts = []
                for u in range(nu):
                    st = st_pool.tile([128, un], F32)
                    for j in range(nj):
                        c0 = u * un + j * 512
                        nc.tensor.matmul(
                            st[:, j * 512 : (j + 1) * 512],
                            kt_t[:d, kc * 128 : (kc + 1) * 128],
                            qt_t[:d, c0 : c0 + 512],
                            start=True,
                            stop=True,
                        )
                    sts.append(st)

                # evictions for kc
                sps = []
                for u in range(nu):
                    m_sl = m_t[:, u * un : (u + 1) * un]
                    sp = sp_pool.tile([128, un], BF16)
                    if kc < n16:
                        se = se_pool.tile([128, un], BF16)
                        nc.scalar.copy(se[:], sts[u][:])
                        nc.vector.tensor_mul(sp[:], se[:], m_sl)
                    elif unit % fb < fa:
                        nc.vector.tensor_mul(sp[:], sts[u][:], m_sl.bitcast(FP8))
                    else:
                        se = se_pool.tile([128, un], BF16)
                        nc.scalar.copy(se[:], sts[u][:])
                        nc.gpsimd.tensor_mul(sp[:], se[:], m_sl.bitcast(FP8))
                    unit += 1
                    sps.append(sp)

                # MM2 groups run `depth` kc-steps behind MM1 (software pipeline)
                hist.append((kc, sps))
                if len(hist) > depth:
                    k0, sp0 = hist.pop(0)
                    for u in range(nu):
                        for j in range(nj):
                            c0 = u * un + j * 512
                            nc.tensor.matmul(
                                ot_ps[:d, c0 : c0 + 512],
                                v_t[:, k0 * d : (k0 + 1) * d],
                                sp0[u][:, j * 512 : (j + 1) * 512],
                                start=(k0 == 0),
                                stop=(k0 == nkc - 1),
                            )

            for k0, sp0 in hist:  # drain
                for u in range(nu):
                    for j in range(nj):
                        c0 = u * un + j * 512
                        nc.tensor.matmul(
                            ot_ps[:d, c0 : c0 + 512],
                            v_t[:, k0 * d : (k0 + 1) * d],
                            sp0[u][:, j * 512 : (j + 1) * 512],
                            start=(k0 == 0),
                            stop=(k0 == nkc - 1),
                        )

            o_t = o_pool.tile([128, sq], BF16)
            nc.scalar.copy(o_t[:d], ot_ps[:d])
            nc.sync.dma_start(out=ot[p], in_=o_t[:d])


def emit_body_v2(
    nc, tc, ot, qt, kt, v, mt, n_pairs, sq, sk, d=D, qn=512, gn=1024, fused_mod=(1, 4),
    repeat=1, loop_n=1,
):
    """Balanced-engine variant.

    Masking is split across three engines per [128, gn] score group:
      - fused path (idx % fused_mod[1] < fused_mod[0]): DVE multiplies
        PSUM f32 scores by the fp8 mask directly -> bf16 SBUF.
      - split path: ACT evicts PSUM f32 -> bf16 SBUF, GpSimd converts the
        fp8 mask -> bf16, DVE multiplies bf16 x bf16 in its 2x mode.
    Second matmul runs with bf16 moving operand at N=gn; V ships as bf16.
    """
    nkc = sk // 128
    ngc = sq // gn

    import contextlib

    with contextlib.ExitStack() as ctx:
        qt_pool = ctx.enter_context(tc.tile_pool(name="qt", bufs=2))
        kt_pool = ctx.enter_context(tc.tile_pool(name="kt", bufs=2))
        v_pool = ctx.enter_context(tc.tile_pool(name="v", bufs=2))
        m_pool = ctx.enter_context(tc.tile_pool(name="m", bufs=4))
        sp_pool = ctx.enter_context(tc.tile_pool(name="sp", bufs=6))
        se_pool = ctx.enter_context(tc.tile_pool(name="se", bufs=4))
        mb_pool = ctx.enter_context(tc.tile_pool(name="mb", bufs=4))
        o_pool = ctx.enter_context(tc.tile_pool(name="o", bufs=2))
        st_pool = ctx.enter_context(tc.tile_pool(name="st", bufs=2, space="PSUM"))
        ot_pool = ctx.enter_context(tc.tile_pool(name="otp", bufs=1, space="PSUM"))

        unit = 0
        loop_cm = tc.For_i(0, loop_n, 1) if loop_n > 1 else contextlib.nullcontext()
        with loop_cm:
          for p in [pp for _ in range(repeat) for pp in range(n_pairs)]:
            qt_t = qt_pool.tile([128, sq], F32R)
            nc.sync.dma_start(out=qt_t[:d], in_=qt[p])
            kt_t = kt_pool.tile([128, sk], F32R)
            nc.sync.dma_start(out=kt_t[:d], in_=kt[p])
            v_t = v_pool.tile([128, nkc * d], BF16)
            nc.sync.dma_start(out=v_t[:], in_=v[p])

            ot_ps = ot_pool.tile([128, sq], F32)

            for kc in range(nkc):
                m_t = m_pool.tile([128, sq], U8)
                nc.sync.dma_start(out=m_t[:], in_=mt[p, kc * 128 : (kc + 1) * 128, :])

                for g in range(ngc):
                    st = st_pool.tile([128, gn], F32)
                    for j in range(gn // qn):
                        c0 = g * gn + j * qn
                        nc.tensor.matmul(
                            st[:, j * qn : (j + 1) * qn],
                            kt_t[:d, kc * 128 : (kc + 1) * 128],
                            qt_t[:d, c0 : c0 + qn],
                            start=True,
                            stop=True,
                        )
                    m_sl = m_t[:, g * gn : (g + 1) * gn].bitcast(FP8)
                    sp = sp_pool.tile([128, gn], BF16)
                    if unit % fused_mod[1] < fused_mod[0]:
                        nc.vector.tensor_mul(sp[:], st[:], m_sl)
                    else:
                        se = se_pool.tile([128, gn], BF16)
                        nc.scalar.copy(se[:], st[:])
                        mb = mb_pool.tile([128, gn], BF16)
                        nc.gpsimd.tensor_copy(mb[:], m_sl)
                        nc.vector.tensor_mul(sp[:], se[:], mb[:])
                    unit += 1
                    for j in range(gn // qn):
                        c0 = g * gn + j * qn
                        nc.tensor.matmul(
                            ot_ps[:d, c0 : c0 + qn],
                            v_t[:, kc * d : (kc + 1) * d],
                            sp[:, j * qn : (j + 1) * qn],
                            start=(kc == 0),
                            stop=(kc == nkc - 1),
                        )

            o_t = o_pool.tile([128, sq], F32)
            nc.scalar.copy(o_t[:d], ot_ps[:d])
            nc.sync.dma_start(out=ot[p], in_=o_t[:d])


def build_nc(n_pairs=PAIRS_PER_CORE, sq=SQ, sk=SK, d=D, qn=None, variant="v1", repeat=1,
             loop_n=1):
    nc = bacc.Bacc("TRN2", target_bir_lowering=False, debug=False)
    if variant.startswith("v4"):
        # v4[_n{n16}][_f{a}_{b}][_d{depth}]  e.g. v4, v4_n8_f1_1_d2
        n16, fa, fb, depth = 4, 4, 7, 1
        toks = variant.split("_")[1:]
        for i, tok in enumerate(toks):
            if tok.startswith("n"):
                n16 = int(tok[1:])
            elif tok.startswith("f"):
                fa = int(tok[1:])
                fb = int(toks[i + 1])
            elif tok.startswith("d"):
                depth = int(tok[1:])
        qt = nc.declare_dram_parameter("qt", [n_pairs, d, sq], BF16, isOutput=False)
        kt = nc.declare_dram_parameter("kt", [n_pairs, d, sk], BF16, isOutput=False)
        v = nc.declare_dram_parameter("v", [n_pairs, 128, (sk // 128) * d], BF16, isOutput=False)
        mt8 = nc.declare_dram_parameter("mt8", [n_pairs, sk, sq], U8, isOutput=False)
        mt16 = nc.declare_dram_parameter("mt16", [n_pairs, max(n16, 1) * 128, sq], BF16, isOutput=False)
        ot = nc.declare_dram_parameter("ot", [n_pairs, d, sq], BF16, isOutput=True)
        with tile.TileContext(nc) as tc:
            emit_body_v4(nc, tc, ot, qt, kt, v, mt8, mt16, n_pairs, sq, sk, d,
                         n16=n16, fa=fa, fb=fb, repeat=repeat, loop_n=loop_n,
                         depth=depth)
        nc.compile()
        return nc
    if variant.startswith("v3"):
        # v3 / v3_bf16 / v3_bf16_512 / v3_512 ...
        parts = variant.split("_")
        mmdt = BF16 if "bf16" in parts else F32R
        if qn is None:
            qn = int(parts[-1]) if parts[-1].isdigit() else 1024
    else:
        mmdt = F32R if variant == "v1" else BF16
        if qn is None:
            qn = 512
    vdt = mmdt
    qt = nc.declare_dram_parameter("qt", [n_pairs, d, sq], mmdt, isOutput=False)
    kt = nc.declare_dram_parameter("kt", [n_pairs, d, sk], mmdt, isOutput=False)
    v = nc.declare_dram_parameter("v", [n_pairs, 128, (sk // 128) * d], vdt, isOutput=False)
    mt = nc.declare_dram_parameter("mt", [n_pairs, sk, sq], U8, isOutput=False)
    ot = nc.declare_dram_parameter("ot", [n_pairs, d, sq], F32, isOutput=True)
    with tile.TileContext(nc) as tc:
        if variant.startswith("v3"):
            emit_body_v3(nc, tc, ot, qt, kt, v, mt, n_pairs, sq, sk, d, qn,
                         repeat=repeat, loop_n=loop_n, mmdt=mmdt)
        elif variant == "v1":
            emit_body(nc, tc, ot, qt, kt, v, mt, n_pairs, sq, sk, d, qn, repeat=repeat,
                      loop_n=loop_n, mmdt=mmdt)
        else:
            emit_body_v2(nc, tc, ot, qt, kt, v, mt, n_pairs, sq, sk, d, qn,
                         repeat=repeat, loop_n=loop_n)
    nc.compile()
    return nc


def _prep_inputs(query, key, value, dropout_mask, variant="v1"):
    """Host-side marshaling into per-core input maps."""
    import ml_dtypes

    q = np.asarray(query, dtype=np.float32).reshape(PAIRS, SQ, D)
    k = np.asarray(key, dtype=np.float32).reshape(PAIRS, SK, D)
    vv = np.asarray(value, dtype=np.float32).reshape(PAIRS, SK, D)
    m = np.asarray(dropout_mask).reshape(PAIRS, SQ, SK)

    qt = np.ascontiguousarray(q.transpose(0, 2, 1))  # [PAIRS, D, SQ]
    kt = np.ascontiguousarray(k.transpose(0, 2, 1))  # [PAIRS, D, SK]
    # V * SCALE rearranged: vr[p][r][c*D+j] = V[c*128+r, j] * SCALE
    vr = (vv * np.float32(SCALE)).reshape(PAIRS, SK // 128, 128, D)
    vr = np.ascontiguousarray(vr.transpose(0, 2, 1, 3)).reshape(PAIRS, 128, (SK // 128) * D)
    if variant.startswith("v4"):
        n16 = 4
        for tok in variant.split("_")[1:]:
            if tok.startswith("n"):
                n16 = int(tok[1:])
        mb = (m != 0).astype(np.uint8) * np.uint8(FP8_ONE)  # [PAIRS, SQ, SK]
        mbt = np.ascontiguousarray(mb.transpose(0, 2, 1))  # [PAIRS, SK, SQ]
        m16 = (m != 0).transpose(0, 2, 1)[:, : max(n16, 1) * 128, :]
        m16 = np.ascontiguousarray(m16).astype(ml_dtypes.bfloat16)
        in_maps = []
        for c in range(N_CORES):
            s = slice(c * PAIRS_PER_CORE, (c + 1) * PAIRS_PER_CORE)
            in_maps.append(
                {
                    "qt": qt[s].astype(ml_dtypes.bfloat16),
                    "kt": kt[s].astype(ml_dtypes.bfloat16),
                    "v": vr[s].astype(ml_dtypes.bfloat16),
                    "mt8": mbt[s],
                    "mt16": m16[s],
                }
            )
        return in_maps
    if variant.startswith("v3"):
        use_bf16 = "bf16" in variant.split("_")
    else:
        use_bf16 = variant != "v1"
    if use_bf16:
        vr = vr.astype(ml_dtypes.bfloat16)
        qt = qt.astype(ml_dtypes.bfloat16)
        kt = kt.astype(ml_dtypes.bfloat16)
    # mask^T as fp8 bytes
    mb = (m != 0).astype(np.uint8) * np.uint8(FP8_ONE)  # [PAIRS, SQ, SK] u8
    mbt = np.ascontiguousarray(mb.transpose(0, 2, 1))  # [PAIRS, SK, SQ]

    in_maps = []
    for c in range(N_CORES):
        s = slice(c * PAIRS_PER_CORE, (c + 1) * PAIRS_PER_CORE)
        in_maps.append(
            {
                "qt": qt[s],
                "kt": kt[s],
                "v": vr[s],
                "mt": mbt[s],
            }
        )
    return in_maps


def kernel(query, key, value, dropout_mask):
    global LAST_RESULTS
    variant = os.environ.get("KERNEL_VARIANT", DEFAULT_VARIANT)
    in_maps = _prep_inputs(query, key, value, dropout_mask, variant)
    nc = build_nc(variant=variant)
    res = run_bass_kernel_spmd(nc, in_maps, list(range(N_CORES)), trace=False)
    LAST_RESULTS = res
    outs = np.concatenate([r["ot"] for r in res.results], axis=0)  # [PAIRS, D, SQ]
    out = outs.astype(np.float32).transpose(0, 2, 1).reshape(B, H, SQ, D)
    return np.ascontiguousarray(out)

